# revision 28
# baseline (speedup 1.0000x reference)
"""Chamfer loss kernel for Trainium2 (8 NeuronCores, data-parallel over batch).

For each batch element b (one per core):
    P[i, j] = ||gts[b, i] - preds[b, j]||^2     (8192 x 8192)
    loss[b] = mean_j min_i P[i, j] + mean_i min_j P[i, j]

Device-side per core:
  - PE computes P in [128 x 2048] fp32 PSUM groups via an augmented matmul.
    To reach ~fp32 accuracy at bf16 PE speed (1 cycle/column vs 4 for fp32),
    every fp32 operand is decomposed into three bf16 terms (hi/lo/lolo) and
    the product expanded into K=24 exact bf16x bf16 partial products:
       W rows (stationary, per gt chunk): -2*g_{hi,lo,lolo} x dims, |g|^2 splits, ones
       X rows (moving, per pred slice):    p_{hi,lo,lolo} x dims, ones, |p|^2 splits
    so (W^T X)[i,j] = |g_i|^2 + |p_j|^2 - 2 g_i.p_j to ~1e-7 absolute.
  - ScalarE evacuates each PSUM group into a full [128, 8192] bf16 slab.
  - VectorE: one bf16 2x-mode tensor_tensor(min) accumulates the column-min
    partials (per pred, min over gt chunks at each partition); the row-min
    (min over preds, loss_2) uses a bf16 2x fold chain (8192->4096->2048->
    1024) plus one 1x tensor_reduce(min).
  - Tail: PE 128x128 transposes of the column-min partials + reduce(min)
    finish min over gt; sums reduced on-device via a matmul with ones.
Output per core: [2, 1] fp32 = (sum of row-mins, sum of col-mins).
Measured on trn2: ~445 us device time for the full 8-core kernel;
loss relative error vs the fp32 jax reference ~4.5e-4.
"""

import numpy as np
import ml_dtypes

import bass_rust
import concourse.bacc as bacc
import concourse.bass as bass
import concourse.masks as masks
import concourse.mybir as mybir
import concourse.tile as tile
from concourse.bass_utils import run_bass_kernel_spmd

F32 = mybir.dt.float32
F32R = mybir.dt.float32r
BF16 = mybir.dt.bfloat16
MIN = mybir.AluOpType.min
FLT_MAX = float(np.finfo(np.float32).max)

B = 8
N_GT = 8192
N_PRED = 8192
N_CORES = 8
MM_FREE = 512           # one PSUM bank of fp32 per matmul
FD_GROUP = 2048         # 4 banks per PSUM group / DVE instruction

_LAST_INFO = {}


def _round_fp32r(x):
    """Round fp32 to the fp32r grid (11-bit mantissa, low 12 bits zero)."""
    b = x.view(np.uint32)
    b = (b + np.uint32(0x800)) & np.uint32(0xFFFFF000)
    return b.view(np.float32)


def _split3(x):
    """x (fp32) ~= hi + lo + lolo, each exactly representable in bf16."""
    hi = x.astype(ml_dtypes.bfloat16).astype(np.float32)
    r = x - hi
    lo = r.astype(ml_dtypes.bfloat16).astype(np.float32)
    lolo = (r - lo).astype(ml_dtypes.bfloat16).astype(np.float32)
    return hi, lo, lolo


def _host_prep(preds, gts, mm_dtype="bf16_split"):
    """Build augmented operands per batch element.

    bf16_split: wt/xt [B, 24, N] bf16. f32r: wt/xt [B, 5, N] fp32(fp32r grid).
    """
    preds = np.asarray(preds, np.float32)
    gts = np.asarray(gts, np.float32)
    g = np.ascontiguousarray(np.swapaxes(gts, 1, 2))    # [B, 3, N_GT]
    p = np.ascontiguousarray(np.swapaxes(preds, 1, 2))  # [B, 3, N_PRED]
    xx = np.sum(g * g, axis=1, keepdims=True)           # [B, 1, N_GT]
    yy = np.sum(p * p, axis=1, keepdims=True)           # [B, 1, N_PRED]
    ones_g = np.ones_like(xx)
    ones_p = np.ones_like(yy)
    if mm_dtype == "f32r":
        wt = np.ascontiguousarray(
            np.concatenate([-2.0 * g, xx, ones_g], axis=1), np.float32)
        xt = np.ascontiguousarray(
            np.concatenate([p, ones_p, yy], axis=1), np.float32)
        return _round_fp32r(wt), _round_fp32r(xt)

    g_hi, g_lo, g_ll = _split3(g)
    p_hi, p_lo, p_ll = _split3(p)
    xx_hi, xx_lo, xx_ll = _split3(xx)
    yy_hi, yy_lo, yy_ll = _split3(yy)
    w_rows, x_rows = [], []
    for d in range(3):
        s = slice(d, d + 1)
        # product pairs: (hi,hi) (hi,lo) (hi,lolo) (lo,hi) (lo,lo) (lolo,hi)
        w_rows += [-2.0 * g_hi[:, s]] * 3 + [-2.0 * g_lo[:, s]] * 2 \
                  + [-2.0 * g_ll[:, s]]
        x_rows += [p_hi[:, s], p_lo[:, s], p_ll[:, s],
                   p_hi[:, s], p_lo[:, s], p_hi[:, s]]
    w_rows += [xx_hi, xx_lo, xx_ll, ones_g, ones_g, ones_g]
    x_rows += [ones_p, ones_p, ones_p, yy_hi, yy_lo, yy_ll]
    wt = np.ascontiguousarray(np.concatenate(w_rows, axis=1))   # [B, 24, N_GT]
    xt = np.ascontiguousarray(np.concatenate(x_rows, axis=1))
    return wt.astype(ml_dtypes.bfloat16), xt.astype(ml_dtypes.bfloat16)


def _legalize_waits(nc):
    """Walrus caps sync waits at 1 per instruction (2 for EventSemaphore).

    Tile can emit more; spill extras onto EventSemaphore instructions
    inserted just before the over-subscribed instruction on the same engine."""
    n_ev = 0
    for blk in nc.m.functions[0].blocks:
        out = []
        changed = False
        for ins in blk.instructions:
            si = ins.sync_info
            waits = list(si.on_wait) if si else []
            cap = 2 if ins.opcode == "EventSemaphore" else 1
            if len(waits) > cap:
                spill, keep = waits[:-cap], waits[-cap:]
                for i in range(0, len(spill), 2):
                    ev = mybir.InstEventSemaphore(
                        name=f"evspill-{n_ev}", ins=[], outs=[])
                    n_ev += 1
                    ev.engine = ins.engine
                    ev.sync_info = bass_rust.SyncInfo(
                        on_wait=spill[i:i + 2], on_update=[])
                    out.append(ev)
                ins.sync_info = bass_rust.SyncInfo(
                    on_wait=keep, on_update=list(si.on_update))
                changed = True
            out.append(ins)
        if changed:
            blk.instructions = out
    return nc


def build_nc(n_gt=N_GT, n_pred=N_PRED, mm_dtype="bf16_split", fd_group=FD_GROUP,
             repeat=1, dve_evac=0):
    """Build the single-core Bacc program (SPMD across cores)."""
    assert n_gt % 128 == 0 and n_pred % fd_group == 0 and fd_group % MM_FREE == 0
    n_ic = n_gt // 128
    n_jg = n_pred // fd_group
    n_blk = n_pred // 128
    mm_per_group = fd_group // MM_FREE
    if mm_dtype == "bf16_split":
        k_aug, sb_dt = 24, BF16
    else:
        k_aug, sb_dt = 5, F32R

    nc = bacc.Bacc()
    wx_d = nc.declare_dram_parameter("wx", [k_aug, n_gt + n_pred], sb_dt,
                                     isOutput=False)
    sums_d = nc.declare_dram_parameter("sums", [2, 1], F32, isOutput=True)

    with tile.TileContext(nc) as tc:
        with (
            tc.tile_pool(name="const", bufs=1) as cpool,
            tc.tile_pool(name="dtiles", bufs=4) as dpool,
            tc.tile_pool(name="rgrp", bufs=4) as rpool,
        ):
            wx_sb = cpool.tile([k_aug, n_gt + n_pred], sb_dt)
            rm_sb = cpool.tile([128, n_ic], F32)
            cm_sb = cpool.tile([128, n_pred], BF16)
            wt_sb = wx_sb[:, :n_gt]
            xt_sb = wx_sb[:, n_gt:]

            nc.gpsimd.dma_start(wx_sb[:], wx_d[:])

            # ---- main sweep over the n_gt x n_pred distance matrix ----
            import contextlib
            rep_ctx = (tc.For_i(0, repeat, 1) if repeat > 1
                       else contextlib.nullcontext())
            with rep_ctx, tc.tile_pool(name="psum", bufs=2, space="PSUM") as ppool:
                for ic in range(n_ic):
                    w_slice = wt_sb[:, ic * 128:(ic + 1) * 128]
                    # slab: the full [128, n_pred] bf16 distance row-block
                    if ic == 0:
                        slab = cm_sb[:]
                    else:
                        slab = dpool.tile([128, n_pred], BF16, tag="dslab")
                    for jg in range(n_jg):
                        ps = ppool.tile([128, fd_group], F32)
                        for k in range(mm_per_group):
                            j0 = jg * fd_group + k * MM_FREE
                            nc.tensor.matmul(
                                ps[:, k * MM_FREE:(k + 1) * MM_FREE],
                                w_slice,
                                xt_sb[:, j0:j0 + MM_FREE],
                                start=True, stop=True,
                            )
                        # evacuation split: ScalarE is the bottleneck engine,
                        # so VectorE (which has slack) takes the last slice
                        j0 = jg * fd_group
                        a = fd_group - dve_evac
                        nc.scalar.copy(slab[:, j0:j0 + a], ps[:, :a])
                        if dve_evac:
                            nc.vector.tensor_copy(
                                slab[:, j0 + a:j0 + fd_group],
                                ps[:, a:fd_group])
                    if ic != 0:
                        # col-min accumulate, one big bf16 2x instruction
                        nc.vector.tensor_tensor(
                            out=cm_sb[:], in0=cm_sb[:], in1=slab, op=MIN)
                    # row-min: bf16 2x fold chain, then one 1x reduce
                    h = n_pred // 2
                    f1 = rpool.tile([128, h], BF16, tag="fold1")
                    nc.vector.tensor_tensor(
                        out=f1[:], in0=slab[:, :h], in1=slab[:, h:], op=MIN)
                    while h > 1024:
                        h //= 2
                        f2 = rpool.tile([128, h], BF16,
                                        tag=f"fold{h}")
                        nc.vector.tensor_tensor(
                            out=f2[:], in0=f1[:, :h], in1=f1[:, h:], op=MIN)
                        f1 = f2
                    nc.vector.tensor_reduce(
                        out=rm_sb[:, ic:ic + 1], in_=f1[:],
                        axis=mybir.AxisListType.X, op=MIN)

            # ---- tail: finish col-min over partitions + on-device sums ----
            with tc.tile_pool(name="psumT", bufs=2, space="PSUM") as tpool:
                ident = cpool.tile([128, 128], BF16)
                masks.make_identity(nc, ident[:])
                cmred = cpool.tile([128, n_blk], F32)
                for blk in range(n_blk):
                    pst = tpool.tile([128, 128], BF16, tag="ptrans")
                    nc.tensor.transpose(
                        pst[:], cm_sb[:, blk * 128:(blk + 1) * 128], ident[:])
                    nc.vector.tensor_reduce(
                        out=cmred[:, blk:blk + 1], in_=pst[:],
                        axis=mybir.AxisListType.X, op=MIN)

                rc = cpool.tile([128, 2], F32)
                nc.vector.tensor_reduce(
                    out=rc[:, 0:1], in_=rm_sb[:],
                    axis=mybir.AxisListType.X, op=mybir.AluOpType.add)
                nc.vector.tensor_reduce(
                    out=rc[:, 1:2], in_=cmred[:],
                    axis=mybir.AxisListType.X, op=mybir.AluOpType.add)
                ones = cpool.tile([128, 1], F32)
                nc.vector.memset(ones[:], 1.0)
                psums = tpool.tile([2, 1], F32, tag="psums")
                nc.tensor.matmul(psums[:], rc[:], ones[:], start=True, stop=True)
                sums_sb = cpool.tile([2, 1], F32)
                nc.vector.tensor_copy(sums_sb[:], psums[:])
                nc.sync.dma_start(sums_d[:], sums_sb[:])
    nc.compile()
    return _legalize_waits(nc)


_NC_CACHE = {}


def _get_nc(key):
    if key not in _NC_CACHE:
        _NC_CACHE[key] = build_nc(*key)
    return _NC_CACHE[key]


def kernel(preds, gts, mm_dtype="bf16_split", trace=False):
    """Full-input kernel: preds [B, N, 3], gts [B, M, 3] -> loss [B] fp32."""
    preds = np.asarray(preds, np.float32)
    gts = np.asarray(gts, np.float32)
    b, n_pred, _ = preds.shape
    _, n_gt, _ = gts.shape
    assert b == N_CORES, f"expected batch {N_CORES}, got {b}"

    wt, xt = _host_prep(preds, gts, mm_dtype)
    nc = _get_nc((n_gt, n_pred, mm_dtype, FD_GROUP))

    wx = np.concatenate([wt, xt], axis=2)
    in_maps = [{"wx": wx[i]} for i in range(b)]
    try:
        res = run_bass_kernel_spmd(nc, in_maps, core_ids=list(range(N_CORES)),
                                   trace=trace)
    except ModuleNotFoundError:
        res = run_bass_kernel_spmd(nc, in_maps, core_ids=list(range(N_CORES)),
                                   trace=False)
    _LAST_INFO.clear()
    _LAST_INFO["exec_time_ns"] = res.exec_time_ns

    out = np.zeros([b], np.float32)
    for i in range(b):
        sums = np.asarray(res.results[i]["sums"], np.float32).reshape(-1)
        loss2 = sums[0] / n_gt      # mean over gts of min over preds
        loss1 = sums[1] / n_pred    # mean over preds of min over gts
        out[i] = loss1 + loss2
    return out
